# revision 15
# baseline (speedup 1.0000x reference)
"""Trainium2 Bass kernel for a 2-layer GAT (nn_GAT_61125974557697).

Strategy (8 NeuronCores, SPMD single program):
  - Host: add self-loops, sort edges by destination, shard destinations
    contiguously across the 8 cores (12500 dst nodes each). Each core's
    dsts are processed in 100 windows of 125 dst nodes. Within a window,
    edges are packed into T tiles of 128 edges (partition dim = edge).
  - Device phase A (replicated): T1[n] = [h1(128) | a_src1(4) | a_dst1(4)]
    = x @ W1ext via PE, where W1ext folds the attention inner products
    into extra output columns (a_src[n,h] = x[n] . (W1_h @ att_src1[h])).
  - Device phase B (layer-1 edge phase, per dst window):
      * indirect-DMA gather of T1[src_e] rows (h|a_src) for T*128 edges
      * indirect-DMA gather of a_dst1[dst_e] (16B rows)
      * e = a_src + a_dst; leaky_relu(0.2); w = exp(e)  (segment-max shift
        is skipped: |e| <= ~10 so exp is safe in fp32; softmax is
        mathematically identical)
      * one-hot selection matrix S[e, dstcol] built on DVE by comparing a
        resident iota row against the per-edge local dst index
      * PE: psum[dst, 0:132] += S^T @ [h*w | w]  accumulated over the T
        edge tiles -> numerators (128) and denominators (4 heads)
      * out1 = num/den; + bias1; ELU; PE-transpose; T2 rows = elu @ W2ext
  - AllGather of the per-core T2 shard -> full T2 table on every core.
  - Device phase C (layer-2 edge phase): same windows, 1 head, 16 channels.
  - Host: concat out2 shards, select the per-graph root rows.

Numerics: fp32 throughout; differences vs the JAX reference are only
summation order and the skipped max-shift (~1e-6 relative).
"""

import numpy as np

import concourse.bass as bass
import concourse.bacc as bacc
import concourse.mybir as mybir
import concourse.tile as tile
from concourse.bass_utils import run_bass_kernel_spmd
from concourse.masks import make_identity

F32 = mybir.dt.float32
I32 = mybir.dt.int32

# Problem constants (hardcoded per spec nn_GAT_61125974557697)
N_FULL = 100000
E_FULL = 1600000
IN_DIM = 128
HEADS = 4
HID = 32
OUT_DIM = 16
G_FULL = 100
NEG_SLOPE = 0.2
N_CORES = 8


class Cfg:
    def __init__(self, N, ncores=N_CORES, wd=125, T=None):
        self.N = N
        self.ncores = ncores
        self.ND = N // ncores          # dst nodes per core
        self.WD = wd                   # dst nodes per window (<=128)
        assert self.ND % wd == 0
        self.NW = self.ND // wd        # windows per core
        self.NT = (N + 127) // 128     # phase-A node tiles
        self.T = T                     # edge tiles (128 edges) per window
        self.NW2 = self.NW             # layer-2 windows actually computed
        self.F1 = IN_DIM + 2 * HEADS   # 136: h(128)|a_src(4)|a_dst(4)
        self.F2 = 20                   # h2(16)|a_src2(1)|a_dst2(1)|pad(2)


# ---------------------------------------------------------------------------
# Host-side preprocessing
# ---------------------------------------------------------------------------

def prep_edges(src, dst, cfg):
    """Sort edges by dst, shard by dst range, pack into [128, NW*T] arrays.

    Element [p, w*T + t] of each per-core array describes edge number
    (t*128 + p) of window w (edges of window w = edges whose dst lies in
    [core*ND + w*WD, core*ND + (w+1)*WD), in dst-sorted order).
    Padding slots: srci=0, dsti=<window's first dst>, dloc=127 (a pad
    column of the selection matrix; its aggregation output is discarded).
    """
    order = np.argsort(dst, kind="stable")
    src_s = src[order].astype(np.int32)
    dst_s = dst[order].astype(np.int32)

    ND, WD, NW = cfg.ND, cfg.WD, cfg.NW
    ncores = cfg.ncores
    # window boundaries over the global sorted dst array
    bounds = np.searchsorted(dst_s, np.arange(0, cfg.N + 1, WD))
    counts = np.diff(bounds)  # edges per window, [ncores*NW]
    tmax = int(np.max((counts + 127) // 128))
    if cfg.T is None:
        cfg.T = max(tmax, 1)
    assert cfg.T >= tmax
    T = cfg.T

    srci = np.zeros((ncores, 128, NW * T), dtype=np.int32)
    dsti = np.zeros((ncores, 128, NW * T), dtype=np.int32)
    dloc = np.full((ncores, 128, NW * T), 127.0, dtype=np.float32)

    for c in range(ncores):
        for w in range(NW):
            gw = c * NW + w
            lo, hi = bounds[gw], bounds[gw + 1]
            cnt = hi - lo
            base = c * ND + w * WD
            # defaults for pad slots in this window's block
            dsti[c, :, w * T:(w + 1) * T] = base
            if cnt == 0:
                continue
            j = np.arange(cnt)
            t = j // 128
            p = j % 128
            col = w * T + t
            srci[c, p, col] = src_s[lo:hi]
            dsti[c, p, col] = dst_s[lo:hi]
            dloc[c, p, col] = (dst_s[lo:hi] - base).astype(np.float32)
    return srci, dsti, dloc


def prep_weights(W1, att_src1, att_dst1, W2, att_src2, att_dst2,
                 bias1, bias2):
    """Fold attention vectors into extra matmul output columns."""
    W1 = np.asarray(W1, np.float32)
    W2 = np.asarray(W2, np.float32)
    a_s1 = np.asarray(att_src1, np.float32)   # [H, HID]
    a_d1 = np.asarray(att_dst1, np.float32)
    a_s2 = np.asarray(att_src2, np.float32)   # [1, OUT]
    a_d2 = np.asarray(att_dst2, np.float32)

    w1ext = np.zeros((IN_DIM, IN_DIM + 2 * HEADS), dtype=np.float32)
    w1ext[:, :IN_DIM] = W1
    for h in range(HEADS):
        blk = W1[:, h * HID:(h + 1) * HID]
        w1ext[:, IN_DIM + h] = blk @ a_s1[h]
        w1ext[:, IN_DIM + HEADS + h] = blk @ a_d1[h]

    w2ext = np.zeros((IN_DIM, 20), dtype=np.float32)
    w2ext[:, :OUT_DIM] = W2
    w2ext[:, OUT_DIM] = W2 @ a_s2[0]
    w2ext[:, OUT_DIM + 1] = W2 @ a_d2[0]

    b1b = np.broadcast_to(np.asarray(bias1, np.float32), (128, IN_DIM)).copy()
    b2b = np.broadcast_to(np.asarray(bias2, np.float32), (128, OUT_DIM)).copy()
    return w1ext, w2ext, b1b, b2b


# ---------------------------------------------------------------------------
# Device program
# ---------------------------------------------------------------------------

def gat_body(tc, ins, outs, cfg):
    """Emit the GAT program. ins/outs: dicts name -> bass.AP."""
    nc = tc.nc
    T, NW, WD, ND, N = cfg.T, cfg.NW, cfg.WD, cfg.ND, cfg.N
    F1, F2 = cfg.F1, cfg.F2
    TE = T * 128

    xT = ins["xT"]
    t1h = nc.dram_tensor("t1tab", [N, F1], F32, kind="Internal")
    t2s = nc.dram_tensor("t2shard", [ND, F2], F32, kind="Internal")
    t2f = nc.dram_tensor("t2full", [N, F2], F32, kind="Internal",
                         addr_space="Shared")

    import contextlib
    stack = contextlib.ExitStack()
    cp = stack.enter_context(tc.tile_pool(name="const", bufs=1))
    wp = stack.enter_context(tc.tile_pool(name="work", bufs=4))
    bp = stack.enter_context(tc.tile_pool(name="big", bufs=4))
    pp = stack.enter_context(tc.tile_pool(name="ps", bufs=2, space="PSUM"))
    pq = stack.enter_context(tc.tile_pool(name="ps1", bufs=1, space="PSUM"))
    pr = stack.enter_context(tc.tile_pool(name="ps2", bufs=2, space="PSUM"))

    # --- resident constants -------------------------------------------------
    w1_sb = cp.tile([128, F1], F32)
    nc.sync.dma_start(out=w1_sb[:, :], in_=ins["w1ext"])
    w2_sb = cp.tile([128, F2], F32)
    nc.sync.dma_start(out=w2_sb[:, :], in_=ins["w2ext"])
    b1_sb = cp.tile([128, IN_DIM], F32)
    nc.sync.dma_start(out=b1_sb[:, :], in_=ins["b1"])
    b2_sb = cp.tile([128, OUT_DIM], F32)
    nc.sync.dma_start(out=b2_sb[:, :], in_=ins["b2"])
    NW2 = cfg.NW2
    srci_sb = cp.tile([128, NW * T], I32)
    nc.sync.dma_start(out=srci_sb[:, :], in_=ins["srci"])
    dloc_sb = cp.tile([128, NW * T], F32)
    nc.sync.dma_start(out=dloc_sb[:, :], in_=ins["dloc"])
    wrows_sb = cp.tile([128, NW], I32)
    nc.sync.dma_start(out=wrows_sb[:, :], in_=ins["wrows"])
    srci2_sb = cp.tile([128, NW2 * T], I32)
    nc.sync.dma_start(out=srci2_sb[:, :], in_=ins["srci2"])
    dloc2_sb = cp.tile([128, NW2 * T], F32)
    nc.sync.dma_start(out=dloc2_sb[:, :], in_=ins["dloc2"])
    wrows2_sb = cp.tile([128, NW2], I32)
    nc.sync.dma_start(out=wrows2_sb[:, :], in_=ins["wrows2"])
    ident = cp.tile([128, 128], F32)
    make_identity(nc, ident[:, :])
    iota_sb = cp.tile([128, 128], F32)
    nc.gpsimd.iota(iota_sb[:, :], [[1, 128]], base=0, channel_multiplier=0,
                   allow_small_or_imprecise_dtypes=True)

    # --- phase A: T1 = x @ W1ext -------------------------------------------
    for i in range(cfg.NT):
        lo = i * 128
        m = min(128, N - lo)
        xt = wp.tile([128, 128], F32, tag="xt")
        nc.sync.dma_start(out=xt[:, :m], in_=xT[:, lo:lo + m])
        pa = pq.tile([128, F1], F32, space="PSUM", tag="pa")
        nc.tensor.matmul(out=pa[:m, :], lhsT=xt[:, :m], rhs=w1_sb[:, :],
                         start=True, stop=True)
        sb = wp.tile([128, F1], F32, tag="t1sb")
        nc.vector.tensor_copy(out=sb[:m, :], in_=pa[:m, :])
        nc.sync.dma_start(out=t1h[lo:lo + m, :], in_=sb[:m, :])

    def build_S(dl):
        S = bp.tile([128, TE], F32, tag="S")
        s3 = S[:, :].rearrange("p (t e) -> p t e", e=128)
        nc.vector.tensor_tensor(
            out=s3,
            in0=iota_sb[:, :].unsqueeze(1).broadcast_to([128, T, 128]),
            in1=dl.to_broadcast([128, T, 128]),
            op=mybir.AluOpType.is_equal)
        return S

    # --- phase B: layer-1 edge aggregation per dst window -------------------
    for w in range(NW):
        idx = srci_sb[:, w * T:(w + 1) * T]
        gath = bp.tile([128, T * 132], F32, tag="gath")
        for t in range(T):
            nc.gpsimd.indirect_dma_start(
                out=gath[:, t * 132:(t + 1) * 132], out_offset=None,
                in_=t1h[:, :],
                in_offset=bass.IndirectOffsetOnAxis(
                    ap=idx[:, t:t + 1], axis=0))
        # a_dst of the window's 128 dst columns (one 16B-row gather)
        adw = wp.tile([128, HEADS], F32, tag="adw")
        nc.gpsimd.indirect_dma_start(
            out=adw[:, :], out_offset=None, in_=t1h[:, :],
            in_offset=bass.IndirectOffsetOnAxis(
                ap=wrows_sb[:, w:w + 1], axis=0),
            element_offset=IN_DIM + HEADS)
        S = build_S(dloc_sb[:, w * T:(w + 1) * T])
        # expand a_dst to edges: adst_e = S_tile @ adw  (via PE transpose)
        adstp = pr.tile([128, T * HEADS], F32, space="PSUM", tag="adstp")
        for t in range(T):
            pt = pr.tile([128, 128], F32, space="PSUM", tag="pt")
            nc.tensor.transpose(out=pt[:, :],
                                in_=S[:, t * 128:(t + 1) * 128],
                                identity=ident[:, :])
            st = wp.tile([128, 128], F32, tag="st")
            nc.vector.tensor_copy(out=st[:, :], in_=pt[:, :])
            nc.tensor.matmul(out=adstp[:, t * HEADS:(t + 1) * HEADS],
                             lhsT=st[:, :], rhs=adw[:, :],
                             start=True, stop=True)

        g3 = gath[:, :].rearrange("p (t f) -> p t f", f=132)
        asrc = g3[:, :, IN_DIM:IN_DIM + HEADS]          # [128,T,4]
        pre = wp.tile([128, T * HEADS], F32, tag="pre")
        p3 = pre[:, :].rearrange("p (t f) -> p t f", f=HEADS)
        nc.vector.tensor_tensor(
            out=p3, in0=asrc,
            in1=adstp[:, :].rearrange("p (t f) -> p t f", f=HEADS),
            op=mybir.AluOpType.add)
        tmp = wp.tile([128, T * HEADS], F32, tag="tmp")
        nc.vector.tensor_scalar_mul(out=tmp[:, :], in0=pre[:, :],
                                    scalar1=NEG_SLOPE)
        nc.vector.tensor_tensor(out=pre[:, :], in0=pre[:, :], in1=tmp[:, :],
                                op=mybir.AluOpType.max)
        # w_e = exp(leaky) written into the gathered tile's a_src columns
        nc.scalar.activation(out=asrc, in_=p3,
                             func=mybir.ActivationFunctionType.Exp)

        # msg = h * w_e  (in place, per head broadcast over 32 channels)
        h4 = g3[:, :, 0:IN_DIM].rearrange("p t (a c) -> p t a c", c=HID)
        wb = g3[:, :, IN_DIM:IN_DIM + HEADS].to_broadcast([128, T, HEADS, HID])
        nc.vector.tensor_tensor(out=h4, in0=h4, in1=wb,
                                op=mybir.AluOpType.mult)

        ps = pp.tile([128, 132], F32, space="PSUM", tag="ps1")
        for t in range(T):
            nc.tensor.matmul(out=ps[:, :], lhsT=S[:, t * 128:(t + 1) * 128],
                             rhs=gath[:, t * 132:(t + 1) * 132],
                             start=(t == 0), stop=(t == T - 1))

        den = wp.tile([128, HEADS], F32, tag="den")
        nc.vector.tensor_scalar_add(out=den[:, :],
                                    in0=ps[:, IN_DIM:IN_DIM + HEADS],
                                    scalar1=1e-16)
        nc.vector.reciprocal(out=den[:, :], in_=den[:, :])
        ow = wp.tile([128, IN_DIM], F32, tag="ow")
        nc.vector.tensor_tensor(
            out=ow[:, :].rearrange("p (a c) -> p a c", c=HID),
            in0=ps[:, 0:IN_DIM].rearrange("p (a c) -> p a c", c=HID),
            in1=den[:, :].to_broadcast([128, HEADS, HID]),
            op=mybir.AluOpType.mult)
        # + bias1, ELU
        nc.vector.tensor_tensor(out=ow[:, :], in0=ow[:, :], in1=b1_sb[:, :],
                                op=mybir.AluOpType.add)
        tneg = wp.tile([128, IN_DIM], F32, tag="tneg")
        nc.vector.tensor_scalar_min(out=tneg[:, :], in0=ow[:, :], scalar1=0.0)
        uexp = wp.tile([128, IN_DIM], F32, tag="uexp")
        nc.scalar.activation(out=uexp[:, :], in_=tneg[:, :],
                             func=mybir.ActivationFunctionType.Exp)
        nc.vector.tensor_scalar(out=ow[:, :], in0=ow[:, :], scalar1=0.0,
                                scalar2=1.0, op0=mybir.AluOpType.max,
                                op1=mybir.AluOpType.subtract)
        nc.vector.tensor_tensor(out=ow[:, :], in0=ow[:, :], in1=uexp[:, :],
                                op=mybir.AluOpType.add)
        # T2 rows for this window: (elu)' @ W2ext  via PE transpose
        pt = pr.tile([128, 128], F32, space="PSUM", tag="pt")
        nc.tensor.transpose(out=pt[:, :], in_=ow[:, :], identity=ident[:, :])
        owT = wp.tile([128, 128], F32, tag="owT")
        nc.vector.tensor_copy(out=owT[:, :], in_=pt[:, :])
        p2 = pq.tile([128, F2], F32, space="PSUM", tag="p2ps2")
        nc.tensor.matmul(out=p2[:, :], lhsT=owT[:, :], rhs=w2_sb[:, :],
                         start=True, stop=True)
        t2sb = wp.tile([128, F2], F32, tag="t2sb")
        nc.vector.tensor_copy(out=t2sb[:, :], in_=p2[:, :])
        nc.sync.dma_start(out=t2s[w * WD:(w + 1) * WD, :], in_=t2sb[:WD, :])

    # --- AllGather the T2 shards -------------------------------------------
    nc.gpsimd.collective_compute(
        "AllGather", mybir.AluOpType.bypass,
        replica_groups=[list(range(cfg.ncores))],
        ins=[t2s[:, :].opt()], outs=[t2f[:, :].opt()])

    # --- phase C: layer-2 edge aggregation (root windows only) -------------
    for w in range(NW2):
        idx = srci2_sb[:, w * T:(w + 1) * T]
        ga = bp.tile([128, T * 17], F32, tag="ga2")
        for t in range(T):
            nc.gpsimd.indirect_dma_start(
                out=ga[:, t * 17:(t + 1) * 17], out_offset=None,
                in_=t2f[:, :],
                in_offset=bass.IndirectOffsetOnAxis(
                    ap=idx[:, t:t + 1], axis=0))
        adw = wp.tile([128, 1], F32, tag="adw2")
        nc.gpsimd.indirect_dma_start(
            out=adw[:, :], out_offset=None, in_=t2f[:, :],
            in_offset=bass.IndirectOffsetOnAxis(
                ap=wrows2_sb[:, w:w + 1], axis=0),
            element_offset=OUT_DIM + 1)
        S = build_S(dloc2_sb[:, w * T:(w + 1) * T])
        adstp = pr.tile([128, T], F32, space="PSUM", tag="adstp")
        for t in range(T):
            pt = pr.tile([128, 128], F32, space="PSUM", tag="pt")
            nc.tensor.transpose(out=pt[:, :],
                                in_=S[:, t * 128:(t + 1) * 128],
                                identity=ident[:, :])
            st = wp.tile([128, 128], F32, tag="st")
            nc.vector.tensor_copy(out=st[:, :], in_=pt[:, :])
            nc.tensor.matmul(out=adstp[:, t:t + 1],
                             lhsT=st[:, :], rhs=adw[:, :],
                             start=True, stop=True)

        g3 = ga[:, :].rearrange("p (t f) -> p t f", f=17)
        asrc = g3[:, :, OUT_DIM:OUT_DIM + 1]            # [128,T,1]
        pre = wp.tile([128, T], F32, tag="pre2")
        p3 = pre[:, :].rearrange("p (t f) -> p t f", f=1)
        nc.vector.tensor_tensor(
            out=p3, in0=asrc,
            in1=adstp[:, :].rearrange("p (t f) -> p t f", f=1),
            op=mybir.AluOpType.add)
        tmp = wp.tile([128, T], F32, tag="tmp2")
        nc.vector.tensor_scalar_mul(out=tmp[:, :], in0=pre[:, :],
                                    scalar1=NEG_SLOPE)
        nc.vector.tensor_tensor(out=pre[:, :], in0=pre[:, :], in1=tmp[:, :],
                                op=mybir.AluOpType.max)
        nc.scalar.activation(out=asrc, in_=p3,
                             func=mybir.ActivationFunctionType.Exp)

        h4 = g3[:, :, 0:OUT_DIM].rearrange("p t (o c) -> p t o c", o=1)
        wb = asrc.to_broadcast([128, T, 1, OUT_DIM])
        nc.vector.tensor_tensor(out=h4, in0=h4, in1=wb,
                                op=mybir.AluOpType.mult)

        ps = pq.tile([128, 17], F32, space="PSUM", tag="p2ps2")
        for t in range(T):
            nc.tensor.matmul(out=ps[:, :], lhsT=S[:, t * 128:(t + 1) * 128],
                             rhs=ga[:, t * 17:(t + 1) * 17],
                             start=(t == 0), stop=(t == T - 1))

        den = wp.tile([128, 1], F32, tag="den2")
        nc.vector.tensor_scalar_add(out=den[:, :],
                                    in0=ps[:, OUT_DIM:OUT_DIM + 1],
                                    scalar1=1e-16)
        nc.vector.reciprocal(out=den[:, :], in_=den[:, :])
        o2 = wp.tile([128, OUT_DIM], F32, tag="o2")
        nc.vector.tensor_tensor(
            out=o2[:, :].rearrange("p (o c) -> p o c", o=1),
            in0=ps[:, 0:OUT_DIM].rearrange("p (o c) -> p o c", o=1),
            in1=den[:, :].to_broadcast([128, 1, OUT_DIM]),
            op=mybir.AluOpType.mult)
        nc.vector.tensor_tensor(out=o2[:, :], in0=o2[:, :], in1=b2_sb[:, :],
                                op=mybir.AluOpType.add)
        nc.sync.dma_start(out=outs["out2"][w * WD:(w + 1) * WD, :],
                          in_=o2[:WD, :])

    stack.close()


def build_program(cfg):
    nc = bacc.Bacc("TRN2", target_bir_lowering=False, num_devices=cfg.ncores)
    T, NW = cfg.T, cfg.NW
    ins = {
        "xT": nc.dram_tensor("xT", [IN_DIM, cfg.N], F32,
                             kind="ExternalInput")[:, :],
        "w1ext": nc.dram_tensor("w1ext", [IN_DIM, cfg.F1], F32,
                                kind="ExternalInput")[:, :],
        "w2ext": nc.dram_tensor("w2ext", [IN_DIM, cfg.F2], F32,
                                kind="ExternalInput")[:, :],
        "b1": nc.dram_tensor("b1", [128, IN_DIM], F32,
                             kind="ExternalInput")[:, :],
        "b2": nc.dram_tensor("b2", [128, OUT_DIM], F32,
                             kind="ExternalInput")[:, :],
        "srci": nc.dram_tensor("srci", [128, NW * T], I32,
                               kind="ExternalInput")[:, :],
        "dloc": nc.dram_tensor("dloc", [128, NW * T], F32,
                               kind="ExternalInput")[:, :],
        "wrows": nc.dram_tensor("wrows", [128, NW], I32,
                                kind="ExternalInput")[:, :],
        "srci2": nc.dram_tensor("srci2", [128, cfg.NW2 * T], I32,
                                kind="ExternalInput")[:, :],
        "dloc2": nc.dram_tensor("dloc2", [128, cfg.NW2 * T], F32,
                                kind="ExternalInput")[:, :],
        "wrows2": nc.dram_tensor("wrows2", [128, cfg.NW2], I32,
                                 kind="ExternalInput")[:, :],
    }
    outs = {
        "out2": nc.dram_tensor("out2", [cfg.NW2 * cfg.WD, OUT_DIM], F32,
                               kind="ExternalOutput")[:, :],
    }
    with tile.TileContext(nc) as tc:
        gat_body(tc, ins, outs, cfg)
    nc.compile()
    return nc


# ---------------------------------------------------------------------------
# Entry point
# ---------------------------------------------------------------------------

def _host_prep(x, edge_index, cfg):
    x = np.asarray(x, np.float32)
    ei = np.asarray(edge_index)
    ar = np.arange(cfg.N, dtype=np.int64)
    src = np.concatenate([ei[0], ar])
    dst = np.concatenate([ei[1], ar])
    srci, dsti, dloc = prep_edges(src, dst, cfg)
    return srci, dsti, dloc


def _wrows(cfg, c, wlist):
    """Per-(dstcol p, window) global T1/T2 row index, [128, len(wlist)]."""
    p = np.minimum(np.arange(128), cfg.WD - 1)[:, None]
    base = (c * cfg.ND + np.asarray(wlist, np.int64) * cfg.WD)[None, :]
    return (base + p).astype(np.int32)


def kernel(x, edge_index, batch, W1, att_src1, att_dst1, bias1,
           W2, att_src2, att_dst2, bias2):
    cfg = Cfg(N_FULL)
    x = np.asarray(x, np.float32)
    srci, dsti, dloc = _host_prep(x, edge_index, cfg)
    w1ext, w2ext, b1b, b2b = prep_weights(
        W1, att_src1, att_dst1, W2, att_src2, att_dst2, bias1, bias2)
    xTc = np.ascontiguousarray(x.T)

    # roots & the layer-2 dst windows that contain them
    batch = np.asarray(batch).astype(np.int64)
    node_types = x[:, 0]
    cand = np.where(node_types == 0, np.arange(cfg.N), cfg.N).astype(np.int64)
    roots = np.full(G_FULL, cfg.N, dtype=np.int64)
    np.minimum.at(roots, batch, cand)
    roots = np.clip(roots, 0, cfg.N - 1)  # mirror jax OOB clamp
    rws = []
    for c in range(cfg.ncores):
        rc = roots[(roots >= c * cfg.ND) & (roots < (c + 1) * cfg.ND)]
        ws = sorted(set(((rc - c * cfg.ND) // cfg.WD).tolist()))
        rws.append(ws)
    cfg.NW2 = max(max(len(ws) for ws in rws), 1)
    for ws in rws:
        while len(ws) < cfg.NW2:
            ws.append(ws[-1] if ws else 0)

    nc = build_program(cfg)
    in_maps = []
    T = cfg.T
    for c in range(cfg.ncores):
        cols = np.concatenate(
            [np.arange(w * T, (w + 1) * T) for w in rws[c]])
        in_maps.append({
            "xT": xTc, "w1ext": w1ext, "w2ext": w2ext, "b1": b1b, "b2": b2b,
            "srci": np.ascontiguousarray(srci[c]),
            "dloc": np.ascontiguousarray(dloc[c]),
            "wrows": _wrows(cfg, c, list(range(cfg.NW))),
            "srci2": np.ascontiguousarray(srci[c][:, cols]),
            "dloc2": np.ascontiguousarray(dloc[c][:, cols]),
            "wrows2": _wrows(cfg, c, rws[c]),
        })
    res = run_bass_kernel_spmd(nc, in_maps, core_ids=list(range(cfg.ncores)))

    out_full = np.zeros((cfg.N, OUT_DIM), np.float32)
    for c in range(cfg.ncores):
        o = res.results[c]["out2"]
        for j, w in enumerate(rws[c]):
            lo = c * cfg.ND + w * cfg.WD
            out_full[lo:lo + cfg.WD] = o[j * cfg.WD:(j + 1) * cfg.WD]
    return out_full[roots].astype(np.float32)
